# revision 1
# baseline (speedup 1.0000x reference)
"""Bilateral filter v7 — host-folded g (E-map) + folded reduces.

Same math/layout as kernel3 (pixel-major im2col, one Derivative_Erf ACT pass,
two in-place DVE mults, two segmented reduces), but: 6 tiles with double-
buffered D/w pools so DMA loads hide under compute, and bf16 reduce outputs
so the reduces run in DVE 2x mode (den cast to f32 for the reciprocal).
"""
from contextlib import ExitStack

import numpy as np
import ml_dtypes

import concourse.bass as bass
import concourse.bacc as bacc
import concourse.tile as tile
from concourse import mybir

F32 = mybir.dt.float32
BF16 = mybir.dt.bfloat16

H, W = 480, 640
NPIX = H * W                  # 307200
PPP = NPIX // 128             # 2400 pixels per partition
NSEG = 6
SEGPX = PPP // NSEG           # 400 px per partition per tile
X = SEGPX * 49                # 19600 free elements per tile
SQRT50 = float(np.sqrt(50.0))
N_CORES = 8
PAD = 3
K = 7


def make_dmapP(img):
    """[128, 49*2400] bf16: partition p, col q*49+d = I_pad(px+d) - I(px) for
    px = 2400p+q, pixel-major im2col of the shift differences."""
    from numpy.lib.stride_tricks import sliding_window_view
    img = np.asarray(img, np.float32)
    Ip = np.zeros((H + 2 * PAD, W + 2 * PAD), np.float32)
    Ip[PAD:PAD + H, PAD:PAD + W] = img
    sw = sliding_window_view(Ip, (H, W))          # (7, 7, H, W)
    Dm = sw.reshape(49, NPIX) - img.reshape(1, NPIX)
    Dt = np.ascontiguousarray(Dm.T).reshape(128, PPP * 49)
    return Dt.astype(ml_dtypes.bfloat16)


def make_emapP(img, g49):
    """[128, 49*2400] bf16 E-map: E = sqrt(D^2 - ln(g_d)/50), so that
    DerivErf(sqrt(50)*E) = (2/sqrt(pi)) * g_d * exp(-50 D^2)."""
    from numpy.lib.stride_tricks import sliding_window_view
    img = np.asarray(img, np.float32)
    Ip = np.zeros((H + 2 * PAD, W + 2 * PAD), np.float32)
    Ip[PAD:PAD + H, PAD:PAD + W] = img
    sw = sliding_window_view(Ip, (H, W))
    Dm = sw.reshape(49, NPIX) - img.reshape(1, NPIX)
    r2 = (-np.log(np.maximum(np.asarray(g49, np.float32), 1e-30)) / 50.0)
    E = np.sqrt(Dm * Dm + r2.reshape(49, 1).astype(np.float32))
    Et = np.ascontiguousarray(E.T).reshape(128, PPP * 49)
    return Et.astype(ml_dtypes.bfloat16)


def make_cimg(img):
    """[128, 2400] f32 partition-major flat image."""
    return np.asarray(img, np.float32).reshape(128, PPP)


def emit(nc, dmap_ap, emap_ap, cimg_ap, out_ap, reps=1, hwloop=False):
    derf = mybir.ActivationFunctionType.Derivative_Erf

    with tile.TileContext(nc) as tc, ExitStack() as ctx:
        singles = ctx.enter_context(tc.tile_pool(name="singles", bufs=1))
        dpool = ctx.enter_context(tc.tile_pool(name="dpool", bufs=2))
        wpool = ctx.enter_context(tc.tile_pool(name="wpool", bufs=2))

        c_t = singles.tile([128, PPP], F32, name="c_t")
        nc.sync.dma_start(out=c_t, in_=cimg_ap)
        den = singles.tile([128, PPP], F32, name="den")
        num = singles.tile([128, PPP], F32, name="num")
        denf = singles.tile([128, PPP], F32, name="denf")

        def body():
            for t in range(NSEG):
                D_t = dpool.tile([128, X], BF16, name="D")
                nc.sync.dma_start(out=D_t,
                                  in_=dmap_ap[:, t * X:(t + 1) * X])
                w_t = wpool.tile([128, X], BF16, name="w")
                nc.sync.dma_start(out=w_t,
                                  in_=emap_ap[:, t * X:(t + 1) * X])
                # w := DerivErf(sqrt50 * E) = (2/sqrt(pi)) g * exp(-50 D^2)
                nc.scalar.activation(out=w_t, in_=w_t, func=derf,
                                     bias=0.0, scale=SQRT50)
                wb = w_t[:]
                w_seg = bass.AP(tensor=wb.tensor, offset=wb.offset,
                                ap=[wb.ap[0], [49, SEGPX], [1, 49]])
                # D := w * D  (in-place; D-slot becomes U = g*wtilde*D)
                nc.vector.tensor_tensor(out=D_t, in0=w_t, in1=D_t,
                                        op=mybir.AluOpType.mult)
                db = D_t[:]
                sl = slice(t * SEGPX, (t + 1) * SEGPX)

                def seg(base, off, n):
                    return bass.AP(tensor=base.tensor,
                                   offset=base.offset + off,
                                   ap=[base.ap[0], [49, SEGPX], [1, n]])

                for base, dst in ((wb, den), (db, num)):
                    # fold 49 -> 25 -> 13 with 2x-mode adds, then short reduce
                    nc.vector.tensor_tensor(out=seg(base, 0, 24),
                                            in0=seg(base, 0, 24),
                                            in1=seg(base, 25, 24),
                                            op=mybir.AluOpType.add)
                    nc.vector.tensor_tensor(out=seg(base, 0, 12),
                                            in0=seg(base, 0, 12),
                                            in1=seg(base, 13, 12),
                                            op=mybir.AluOpType.add)
                    nc.vector.tensor_reduce(out=dst[:, sl],
                                            in_=seg(base, 0, 13),
                                            axis=mybir.AxisListType.X,
                                            op=mybir.AluOpType.add)

            # out = c + num/den
            nc.vector.reciprocal_approx_fast(out=denf, in_=den)
            nc.vector.tensor_tensor(out=denf, in0=num, in1=denf,
                                    op=mybir.AluOpType.mult)
            nc.vector.tensor_tensor(out=c_t, in0=denf, in1=c_t,
                                    op=mybir.AluOpType.add)
            ofl = bass.AP(tensor=out_ap.tensor, offset=out_ap.offset,
                          ap=[[PPP, 128], [1, PPP]])
            nc.sync.dma_start(out=ofl, in_=c_t)

        if hwloop and reps > 1:
            with tc.For_i(0, reps):
                body()
        else:
            for _ in range(reps):
                body()


def build_nc(reps=1, hwloop=False):
    nc = bacc.Bacc(num_devices=N_CORES)
    dmap = nc.dram_tensor("dmap", [128, PPP * 49], BF16, kind="ExternalInput")
    emap = nc.dram_tensor("emap", [128, PPP * 49], BF16, kind="ExternalInput")
    cimg = nc.dram_tensor("cimg", [128, PPP], F32, kind="ExternalInput")
    out = nc.dram_tensor("out", [H, W], F32, kind="ExternalOutput")
    emit(nc, dmap.ap(), emap.ap(), cimg.ap(), out.ap(), reps=reps,
         hwloop=hwloop)
    nc.finalize()
    return nc


def make_in_maps(I, g49):
    in_maps = []
    for c in range(I.shape[0]):
        img = I[c, 0]
        in_maps.append({"dmap": make_dmapP(img), "cimg": make_cimg(img),
                        "emap": make_emapP(img, g49)})
    return in_maps


def kernel(I: np.ndarray, g: np.ndarray) -> np.ndarray:
    from concourse.bass_utils import run_bass_kernel_spmd

    I = np.ascontiguousarray(np.asarray(I, np.float32))
    g49 = np.asarray(g, np.float32).reshape(-1)
    nc = build_nc()
    in_maps = make_in_maps(I, g49)
    res = run_bass_kernel_spmd(nc, in_maps, core_ids=list(range(N_CORES)))
    return np.stack([r["out"] for r in res.results], axis=0)



# revision 8
# speedup vs baseline: 2.0000x; 2.0000x over previous
"""Bilateral filter v8 — full on-device stencil, PE-accumulated.

Computes the 7x7 bilateral directly from the padded image (no im2col, no
precomputed D/E maps: ~60MB -> ~10MB of DMA):
  - the 7 dy row-shifts are materialized as strided DMA replicas of one
    flat padded host array (engines cannot read at non-quadrant partition
    bases, DMA can gather anything from DRAM); two column-parity copies
    (A/B) keep every DVE read 4B-aligned for the bf16 2x mode
  - D = S - C on DVE (bf16 2x), w = DerivErf(sqrt50*D) on ACT fuses
    square+exp, T = w*D on DVE
  - +/-d pair symmetry halves the ACT work: w_{-d}(p) = w_d(p-d) and
    w_{-d}(p)*D_{-d}(p) = -T_d(p-d), so each of the 24 positive offsets
    serves both directions via shifted reads
  - accumulation over offsets runs on the Tensor engine: PSUM-accumulated
    matmuls whose stationaries are shifted diagonals +/-g_d*delta(k=j+3-dy)
    (spatial weight and the dy read-shift folded in for free; the dx
    read-shift is a free-axis offset on the moving operand)
  - out = C + numD/(den + g0), reciprocal_approx_fast on DVE

Layout: padded image Ipad[486, 646] in segment tiles R[dy][par]: partition
p, segment k, col c -> Ipad[120k + p + dy, c + (0|1)], rows stored 656 wide
(4-col guard + 646 data + 6 guard) so every +/-3 col shift is in-bounds.
"""
from contextlib import ExitStack

import numpy as np
import ml_dtypes

import concourse.bass as bass
import concourse.bacc as bacc
import concourse.tile as tile
from concourse import mybir

F32 = mybir.dt.float32
BF16 = mybir.dt.bfloat16

H, W = 480, 640
PAD = 3
N_CORES = 8
SEG = 4              # row segments
RPS = 120            # output rows per segment
L = 656              # stored row length: 4 guard + 646 data + 6 guard
HP = 486             # padded rows
WP = 646             # padded cols
IH_ROWS = 497        # 1 guard row + 486 padded + 10 guard
SQRT50 = float(np.sqrt(50.0))

# positive half of the 7x7 offset set (24 offsets; negatives via symmetry)
PAIRS = [(dy, dx) for dy in range(0, 4) for dx in range(-3, 4)
         if dy > 0 or (dy == 0 and dx > 0)]
assert len(PAIRS) == 24


def make_IH(img):
    """[497, 656] bf16: guard row, then Ipad at col offset 4."""
    ih = np.zeros((IH_ROWS, L), np.float32)
    ih[1 + PAD:1 + PAD + H, 4 + PAD:4 + PAD + W] = np.asarray(img, np.float32)
    return ih.astype(ml_dtypes.bfloat16)


def make_G(g49):
    """[128, 72, 120] bf16 stationaries per pair i (PE reads partition-0
    based, so the dy shift of the paired stream lives in the diagonal):
      3i+0: g_d * delta(k = j+3)       (unshifted streams)
      3i+1: g_d * delta(k = j+3-dy)    (shifted den stream)
      3i+2: -g_d * delta(k = j+3-dy)   (shifted num stream)"""
    g49 = np.asarray(g49, np.float32).reshape(-1)
    G = np.zeros((128, 72, 120), np.float32)
    j = np.arange(120)
    for i, (dy, dx) in enumerate(PAIRS):
        g = float(g49[(dy + 3) * 7 + (dx + 3)])
        G[j + 3, 3 * i + 0, j] = g
        G[j + 3 - dy, 3 * i + 1, j] = g
        G[j + 3 - dy, 3 * i + 2, j] = -g
    return G.astype(ml_dtypes.bfloat16)


def emit(nc, IH_ap, G_ap, out_ap, g0=1.0, reps=1, hwloop=False):
    derf = mybir.ActivationFunctionType.Derivative_Erf

    def ih_src(dy, par):
        # R[dy][par][p, k, c] = IH[1 + 120k + p + dy, c - par]
        off = (1 + dy) * L - par
        return bass.AP(tensor=IH_ap.tensor, offset=IH_ap.offset + off,
                       ap=[[L, 128], [RPS * L, SEG], [1, L]])

    with tile.TileContext(nc) as tc, ExitStack() as ctx:
        singles = ctx.enter_context(tc.tile_pool(name="singles", bufs=1))
        dpool = ctx.enter_context(tc.tile_pool(name="dpool", bufs=3))
        wpool = ctx.enter_context(tc.tile_pool(name="wpool", bufs=3))
        tpool = ctx.enter_context(tc.tile_pool(name="tpool", bufs=3))
        ppool = ctx.enter_context(tc.tile_pool(name="ppool", bufs=1,
                                               space="PSUM"))
        opool = ctx.enter_context(tc.tile_pool(name="opool", bufs=2))

        # R[dy][0] = B copy (data at col 4), R[dy][1] = A copy (col 5)
        R = [[singles.tile([128, SEG, L], BF16, name=f"R{dy}{par}")
              for par in range(2)] for dy in range(4)]
        G_t = singles.tile([128, 72, 120], BF16, name="G_t")

        def body():
            for dy in range(4):
                for par in range(2):
                    nc.sync.dma_start(out=R[dy][par], in_=ih_src(dy, par))
            nc.sync.dma_start(out=G_t, in_=G_ap)
            C_B = R[0][0]
            for h in range(2):
                num = [ppool.tile([120, 640], F32, name=f"num{m}")
                       for m in range(2)]
                den = [ppool.tile([120, 640], F32, name=f"den{m}")
                       for m in range(2)]
                for i, (dy, dx) in enumerate(PAIRS):
                    # S read: even dx from B copy (off 4+dx), odd from A (5+dx)
                    par = 1 if dx % 2 else 0
                    c0 = 4 + par + dx
                    D_t = dpool.tile([128, 2, L], BF16, name="D")
                    nc.vector.tensor_tensor(
                        out=D_t[0:123, :, 4:650],
                        in0=R[dy][par][0:123, 2 * h:2 * h + 2, c0:c0 + 646],
                        in1=C_B[0:123, 2 * h:2 * h + 2, 4:650],
                        op=mybir.AluOpType.subtract)
                    W_t = wpool.tile([128, 2, L], BF16, name="Wt")
                    nc.scalar.activation(
                        out=W_t[0:123, :, 4:650], in_=D_t[0:123, :, 4:650],
                        func=derf, bias=0.0, scale=SQRT50)
                    T_t = tpool.tile([128, 2, L], BF16, name="Tt")
                    nc.vector.tensor_tensor(
                        out=T_t[0:123, :, 4:650], in0=W_t[0:123, :, 4:650],
                        in1=D_t[0:123, :, 4:650], op=mybir.AluOpType.mult)
                    gp0 = G_t[0:123, 3 * i + 0, :]
                    gps = G_t[0:123, 3 * i + 1, :]
                    gns = G_t[0:123, 3 * i + 2, :]
                    first, last = (i == 0), (i == 23)
                    for m in range(2):
                        for x0, n in ((0, 512), (512, 128)):
                            nc.tensor.matmul(
                                num[m][:, x0:x0 + n], gp0,
                                T_t[0:123, m, 7 + x0:7 + x0 + n],
                                start=first, stop=False)
                            nc.tensor.matmul(
                                num[m][:, x0:x0 + n], gns,
                                T_t[0:123, m, 7 - dx + x0:7 - dx + x0 + n],
                                start=False, stop=last)
                            nc.tensor.matmul(
                                den[m][:, x0:x0 + n], gp0,
                                W_t[0:123, m, 7 + x0:7 + x0 + n],
                                start=first, stop=False)
                            nc.tensor.matmul(
                                den[m][:, x0:x0 + n], gps,
                                W_t[0:123, m, 7 - dx + x0:7 - dx + x0 + n],
                                start=False, stop=last)
                for m in range(2):
                    rt = 2 * h + m
                    denf = opool.tile([120, 640], F32, name="denf")
                    nc.vector.tensor_scalar(
                        out=denf, in0=den[m][:], scalar1=g0, scalar2=None,
                        op0=mybir.AluOpType.add)
                    rec = opool.tile([120, 640], F32, name="rec")
                    nc.vector.reciprocal_approx_fast(out=rec, in_=denf)
                    q = opool.tile([120, 640], F32, name="q")
                    nc.vector.tensor_tensor(out=q, in0=num[m][:], in1=rec,
                                            op=mybir.AluOpType.mult)
                    o_t = opool.tile([120, 640], F32, name="o")
                    # C on the output rows: Ipad[120rt + 3 + j, 3 + x]
                    nc.vector.tensor_tensor(
                        out=o_t, in0=q, in1=R[3][0][0:120, rt, 7:647],
                        op=mybir.AluOpType.add)
                    nc.sync.dma_start(
                        out=out_ap[120 * rt:120 * rt + 120, :], in_=o_t)

        if hwloop and reps > 1:
            with tc.For_i(0, reps):
                body()
        else:
            for _ in range(reps):
                body()


def build_nc(reps=1, hwloop=False, g0=1.0):
    nc = bacc.Bacc(num_devices=N_CORES)
    IH = nc.dram_tensor("IH", [IH_ROWS, L], BF16, kind="ExternalInput")
    G = nc.dram_tensor("G", [128, 72, 120], BF16, kind="ExternalInput")
    out = nc.dram_tensor("out", [H, W], F32, kind="ExternalOutput")
    emit(nc, IH.ap(), G.ap(), out.ap(), g0=g0, reps=reps, hwloop=hwloop)
    nc.finalize()
    return nc


def make_in_maps(I, g49):
    G = make_G(g49)
    return [{"IH": make_IH(I[c, 0]), "G": G} for c in range(I.shape[0])]


def kernel(I: np.ndarray, g: np.ndarray) -> np.ndarray:
    from concourse.bass_utils import run_bass_kernel_spmd

    I = np.ascontiguousarray(np.asarray(I, np.float32))
    g49 = np.asarray(g, np.float32).reshape(-1)
    nc = build_nc(g0=float(g49[24]))
    in_maps = make_in_maps(I, g49)
    res = run_bass_kernel_spmd(nc, in_maps, core_ids=list(range(N_CORES)))
    return np.stack([r["out"] for r in res.results], axis=0)


# revision 13
# speedup vs baseline: 2.1274x; 1.0637x over previous
"""Bilateral filter v8 — full on-device stencil, PE-accumulated.

Computes the 7x7 bilateral directly from the padded image (no im2col, no
precomputed D/E maps: ~60MB -> ~10MB of DMA):
  - the 7 dy row-shifts are materialized as strided DMA replicas of one
    flat padded host array (engines cannot read at non-quadrant partition
    bases, DMA can gather anything from DRAM); two column-parity copies
    (A/B) keep every DVE read 4B-aligned for the bf16 2x mode
  - D = S - C on DVE (bf16 2x), w = DerivErf(sqrt50*D) on ACT fuses
    square+exp, T = w*D on DVE
  - +/-d pair symmetry halves the ACT work: w_{-d}(p) = w_d(p-d) and
    w_{-d}(p)*D_{-d}(p) = -T_d(p-d), so each of the 24 positive offsets
    serves both directions via shifted reads
  - accumulation over offsets runs on the Tensor engine: PSUM-accumulated
    matmuls whose stationaries are shifted diagonals +/-g_d*delta(k=j+3-dy)
    (spatial weight and the dy read-shift folded in for free; the dx
    read-shift is a free-axis offset on the moving operand)
  - out = C + numD/(den + g0), reciprocal_approx_fast on DVE

Layout: padded image Ipad[486, 646] in segment tiles R[dy][par]: partition
p, segment k, col c -> Ipad[120k + p + dy, c + (0|1)], rows stored 656 wide
(4-col guard + 646 data + 6 guard) so every +/-3 col shift is in-bounds.
"""
from contextlib import ExitStack

import numpy as np
import ml_dtypes

import concourse.bass as bass
import concourse.bacc as bacc
import concourse.tile as tile
from concourse import mybir

F32 = mybir.dt.float32
BF16 = mybir.dt.bfloat16

H, W = 480, 640
PAD = 3
N_CORES = 8
SEG = 4              # row segments
RPS = 120            # output rows per segment
L = 656              # stored row length: 4 guard + 646 data + 6 guard
HP = 486             # padded rows
WP = 646             # padded cols
IH_ROWS = 497        # 1 guard row + 486 padded + 10 guard
SQRT50 = float(np.sqrt(50.0))

# positive half of the 7x7 offset set (24 offsets; negatives via symmetry)
PAIRS = [(dy, dx) for dy in range(0, 4) for dx in range(-3, 4)
         if dy > 0 or (dy == 0 and dx > 0)]
assert len(PAIRS) == 24


def make_IH(img):
    """[497, 656] bf16: guard row, then Ipad at col offset 4."""
    ih = np.zeros((IH_ROWS, L), np.float32)
    ih[1 + PAD:1 + PAD + H, 4 + PAD:4 + PAD + W] = np.asarray(img, np.float32)
    return ih.astype(ml_dtypes.bfloat16)


def make_G(g49):
    """[128, 72, 120] bf16 stationaries per pair i (PE reads partition-0
    based, so the dy shift of the paired stream lives in the diagonal):
      3i+0: g_d * delta(k = j+3)       (unshifted streams)
      3i+1: g_d * delta(k = j+3-dy)    (shifted den stream)
      3i+2: -g_d * delta(k = j+3-dy)   (shifted num stream)"""
    g49 = np.asarray(g49, np.float32).reshape(-1)
    G = np.zeros((128, 72, 120), np.float32)
    j = np.arange(120)
    for i, (dy, dx) in enumerate(PAIRS):
        g = float(g49[(dy + 3) * 7 + (dx + 3)])
        if dx == 0:
            # both streams read the same rhs -> merge into 2-banded matrices
            G[j + 3, 3 * i + 1, j] += g       # den: delta(j+3)+delta(j+3-dy)
            G[j + 3 - dy, 3 * i + 1, j] += g
            G[j + 3, 3 * i + 2, j] += g       # num: delta(j+3)-delta(j+3-dy)
            G[j + 3 - dy, 3 * i + 2, j] += -g
        else:
            G[j + 3, 3 * i + 0, j] = g
            G[j + 3 - dy, 3 * i + 1, j] = g
            G[j + 3 - dy, 3 * i + 2, j] = -g
    return G.astype(ml_dtypes.bfloat16)


def emit(nc, IH_ap, G_ap, out_ap, g0=1.0, reps=1, hwloop=False):
    derf = mybir.ActivationFunctionType.Derivative_Erf

    def ih_src(dy, par):
        # R[dy][par][p, k, c] = IH[1 + 120k + p + dy, c - par]
        off = (1 + dy) * L - par
        return bass.AP(tensor=IH_ap.tensor, offset=IH_ap.offset + off,
                       ap=[[L, 128], [RPS * L, SEG], [1, L]])

    with tile.TileContext(nc) as tc, ExitStack() as ctx:
        singles = ctx.enter_context(tc.tile_pool(name="singles", bufs=1))
        dpool = ctx.enter_context(tc.tile_pool(name="dpool", bufs=3))
        wpool = ctx.enter_context(tc.tile_pool(name="wpool", bufs=3))
        tpool = ctx.enter_context(tc.tile_pool(name="tpool", bufs=3))
        ppool = ctx.enter_context(tc.tile_pool(name="ppool", bufs=1,
                                               space="PSUM"))
        opool = ctx.enter_context(tc.tile_pool(name="opool", bufs=2))

        # R[dy][0] = B copy (data at col 4), R[dy][1] = A copy (col 5)
        R = [[singles.tile([128, SEG, L], BF16, name=f"R{dy}{par}")
              for par in range(2)] for dy in range(4)]
        G_t = singles.tile([128, 72, 120], BF16, name="G_t")

        def body():
            # spread the replica loads over several engines' DMA queues
            dma_engines = [nc.sync, nc.gpsimd, nc.scalar]
            nc.sync.dma_start(out=G_t, in_=G_ap)
            for dy in range(4):
                for par in range(2):
                    eng = dma_engines[(dy * 2 + par) % len(dma_engines)]
                    eng.dma_start(out=R[dy][par], in_=ih_src(dy, par))
            C_B = R[0][0]
            for h in range(2):
                num = [ppool.tile([120, 640], F32, name=f"num{m}")
                       for m in range(2)]
                den = [ppool.tile([120, 640], F32, name=f"den{m}")
                       for m in range(2)]
                for i, (dy, dx) in enumerate(PAIRS):
                    # S read: even dx from B copy (off 4+dx), odd from A (5+dx)
                    par = 1 if dx % 2 else 0
                    c0 = 4 + par + dx
                    D_t = dpool.tile([128, 2, L], BF16, name="D")
                    nc.vector.tensor_tensor(
                        out=D_t[0:123, :, 4:650],
                        in0=R[dy][par][0:123, 2 * h:2 * h + 2, c0:c0 + 646],
                        in1=C_B[0:123, 2 * h:2 * h + 2, 4:650],
                        op=mybir.AluOpType.subtract)
                    W_t = wpool.tile([128, 2, L], BF16, name="Wt")
                    nc.scalar.activation(
                        out=W_t[0:123, :, 4:650], in_=D_t[0:123, :, 4:650],
                        func=derf, bias=0.0, scale=SQRT50)
                    T_t = tpool.tile([128, 2, L], BF16, name="Tt")
                    nc.vector.tensor_tensor(
                        out=T_t[0:123, :, 4:650], in0=W_t[0:123, :, 4:650],
                        in1=D_t[0:123, :, 4:650], op=mybir.AluOpType.mult)
                    gp0 = G_t[0:123, 3 * i + 0, :]
                    gps = G_t[0:123, 3 * i + 1, :]
                    gns = G_t[0:123, 3 * i + 2, :]
                    first, last = (i == 0), (i == 23)
                    CH = ((0, 512), (512, 128))
                    # group matmuls by stationary to avoid reload thrash
                    if dx == 0:
                        # merged 2-banded stationaries, single stream each
                        for m in range(2):
                            for x0, n in CH:
                                nc.tensor.matmul(
                                    den[m][:, x0:x0 + n], gps,
                                    W_t[0:123, m, 7 + x0:7 + x0 + n],
                                    start=False, stop=False)
                        for m in range(2):
                            for x0, n in CH:
                                nc.tensor.matmul(
                                    num[m][:, x0:x0 + n], gns,
                                    T_t[0:123, m, 7 + x0:7 + x0 + n],
                                    start=False, stop=False)
                    else:
                        for m in range(2):
                            for x0, n in CH:
                                nc.tensor.matmul(
                                    num[m][:, x0:x0 + n], gp0,
                                    T_t[0:123, m, 7 + x0:7 + x0 + n],
                                    start=first, stop=False)
                                nc.tensor.matmul(
                                    den[m][:, x0:x0 + n], gp0,
                                    W_t[0:123, m, 7 + x0:7 + x0 + n],
                                    start=first, stop=False)
                        for m in range(2):
                            for x0, n in CH:
                                nc.tensor.matmul(
                                    den[m][:, x0:x0 + n], gps,
                                    W_t[0:123, m, 7 - dx + x0:7 - dx + x0 + n],
                                    start=False, stop=last)
                        for m in range(2):
                            for x0, n in CH:
                                nc.tensor.matmul(
                                    num[m][:, x0:x0 + n], gns,
                                    T_t[0:123, m, 7 - dx + x0:7 - dx + x0 + n],
                                    start=False, stop=last)
                # drain PSUM first (frees banks for the next half's matmuls),
                # then finish the math out of SBUF
                denf, numf = [], []
                for m in range(2):
                    df = opool.tile([120, 640], F32, name=f"denf{m}")
                    nc.vector.tensor_scalar(
                        out=df, in0=den[m][:], scalar1=g0, scalar2=None,
                        op0=mybir.AluOpType.add)
                    nf = opool.tile([120, 640], F32, name=f"numf{m}")
                    nc.scalar.copy(out=nf, in_=num[m][:])
                    denf.append(df)
                    numf.append(nf)
                for m in range(2):
                    rt = 2 * h + m
                    rec = opool.tile([120, 640], F32, name="rec")
                    nc.vector.reciprocal_approx_fast(out=rec, in_=denf[m])
                    q = opool.tile([120, 640], F32, name="q")
                    nc.vector.tensor_tensor(out=q, in0=numf[m], in1=rec,
                                            op=mybir.AluOpType.mult)
                    o_t = opool.tile([120, 640], F32, name="o")
                    # C on the output rows: Ipad[120rt + 3 + j, 3 + x]
                    nc.gpsimd.tensor_tensor(
                        out=o_t, in0=q, in1=R[3][0][0:120, rt, 7:647],
                        op=mybir.AluOpType.add)
                    nc.sync.dma_start(
                        out=out_ap[120 * rt:120 * rt + 120, :], in_=o_t)

        if hwloop and reps > 1:
            with tc.For_i(0, reps):
                body()
        else:
            for _ in range(reps):
                body()


def build_nc(reps=1, hwloop=False, g0=1.0):
    nc = bacc.Bacc(num_devices=N_CORES)
    IH = nc.dram_tensor("IH", [IH_ROWS, L], BF16, kind="ExternalInput")
    G = nc.dram_tensor("G", [128, 72, 120], BF16, kind="ExternalInput")
    out = nc.dram_tensor("out", [H, W], F32, kind="ExternalOutput")
    emit(nc, IH.ap(), G.ap(), out.ap(), g0=g0, reps=reps, hwloop=hwloop)
    nc.finalize()
    return nc


def make_in_maps(I, g49):
    G = make_G(g49)
    return [{"IH": make_IH(I[c, 0]), "G": G} for c in range(I.shape[0])]


def kernel(I: np.ndarray, g: np.ndarray) -> np.ndarray:
    from concourse.bass_utils import run_bass_kernel_spmd

    I = np.ascontiguousarray(np.asarray(I, np.float32))
    g49 = np.asarray(g, np.float32).reshape(-1)
    nc = build_nc(g0=float(g49[24]))
    in_maps = make_in_maps(I, g49)
    res = run_bass_kernel_spmd(nc, in_maps, core_ids=list(range(N_CORES)))
    return np.stack([r["out"] for r in res.results], axis=0)
